# revision 3
# baseline (speedup 1.0000x reference)
"""HDC embedding lookup kernel for Trainium2 (8 NeuronCores).

Strategy: replicate the vocab table, data-parallel shard the 8192 tokens
across 8 cores (1024 tokens each). Per core, raw-Bass pipeline:

  - one DMA loads the core's tokens as a [128, 8] SBUF tile
  - 8 indirect DMAs (SWDGE) gather [128, 10000] vocab rows with an
    f32 -> narrow dtype cast applied in the DMA datapath; gather g uses
    token column g, so its partition p corresponds to shard row p*8+g
  - 8 HWDGE stores write each gathered tile to out rows p*8+g

The kernel is HBM-bandwidth bound (~390 GB/s combined R+W per core); the
f32 read of the gathered rows (40.96 MB/core) is irreducible, so the win
comes from narrowing the store side. The vocab table here is binary
(0.0/1.0 sparse HDC codebook), so a u8 output is bit-exact after a host
astype back to f32; if a non-binary table is ever passed, we fall back to
bf16 output (exact for 0/1; <=0.4% rel error for arbitrary data).

Synchronization note: DMA completion semaphores are per buffer slot, and
every wait threshold equals the maximum count that semaphore can have at
that point. A single aggregate semaphore would be racy: the 16 SDMA
engines complete their slices of consecutive DMAs with per-engine skew,
so an aggregate count of (g+1)*16 does NOT imply gather g fully landed.
With per-slot semaphores the waited value is only reachable when every
engine has finished every DMA issued on that slot.
"""

import numpy as np

from concourse import bass, mybir
from concourse.bass_utils import run_bass_kernel_spmd

N_CORES = 8
VOCAB = 32000
DIM = 10000
N_TOKENS = 8192
TOK_PER_CORE = N_TOKENS // N_CORES  # 1024
P = 128
N_TILES = TOK_PER_CORE // P  # 8
NBUF = 4

_NC_CACHE = {}


def _build_nc(out_dt):
    nc = bass.Bass()
    tokens = nc.dram_tensor(
        "tokens", [TOK_PER_CORE], mybir.dt.int32, kind="ExternalInput"
    )
    vocab = nc.dram_tensor(
        "hdc_vocab", [VOCAB, DIM], mybir.dt.float32, kind="ExternalInput"
    )
    out = nc.dram_tensor(
        "out", [TOK_PER_CORE, DIM], out_dt, kind="ExternalOutput"
    )

    with (
        nc.sbuf_tensor("idx", [P, N_TILES], mybir.dt.int32) as idx,
        nc.sbuf_tensor("rows0", [P, DIM], out_dt) as rows0,
        nc.sbuf_tensor("rows1", [P, DIM], out_dt) as rows1,
        nc.sbuf_tensor("rows2", [P, DIM], out_dt) as rows2,
        nc.sbuf_tensor("rows3", [P, DIM], out_dt) as rows3,
        nc.semaphore("idx_sem") as idx_sem,
        nc.semaphore("gsem0") as gsem0,
        nc.semaphore("gsem1") as gsem1,
        nc.semaphore("gsem2") as gsem2,
        nc.semaphore("gsem3") as gsem3,
        nc.semaphore("ssem0") as ssem0,
        nc.semaphore("ssem1") as ssem1,
        nc.semaphore("ssem2") as ssem2,
        nc.semaphore("ssem3") as ssem3,
        nc.Block() as block,
    ):
        rows = [rows0, rows1, rows2, rows3]
        gsem = [gsem0, gsem1, gsem2, gsem3]
        ssem = [ssem0, ssem1, ssem2, ssem3]

        @block.gpsimd
        def _(gpsimd):
            # tokens [1024] -> [128, 8]: partition p gets 32 contiguous bytes
            gpsimd.dma_start(
                idx[:, :], tokens[:].rearrange("(p t) -> p t", p=P)
            ).then_inc(idx_sem, 16)
            gpsimd.wait_ge(idx_sem, 16)
            for g in range(N_TILES):
                s = g % NBUF
                if g >= NBUF:
                    # rows[s] reuse: store of tile g-NBUF must be fully done
                    gpsimd.wait_ge(ssem[s], (g // NBUF) * 16)
                gpsimd.indirect_dma_start(
                    out=rows[s][:, :],
                    out_offset=None,
                    in_=vocab[:, :],
                    in_offset=bass.IndirectOffsetOnAxis(ap=idx[:, g : g + 1], axis=0),
                ).then_inc(gsem[s], 16)

        @block.sync
        def _(sync):
            for g in range(N_TILES):
                s = g % NBUF
                sync.wait_ge(gsem[s], (g // NBUF + 1) * 16)
                # out rows p*8+g for p in 0..127: offset g rows, stride 8 rows
                sync.dma_start(
                    bass.AP(out, g * DIM, [[N_TILES * DIM, P], [1, DIM]]),
                    rows[s][:, :],
                ).then_inc(ssem[s], 16)

    return nc


def _get_nc(out_dt):
    key = str(out_dt)
    if key not in _NC_CACHE:
        _NC_CACHE[key] = _build_nc(out_dt)
    return _NC_CACHE[key]


def _is_binary(vocab):
    # chunked to keep peak host memory small
    flat = vocab.reshape(-1)
    step = 1 << 24
    for i in range(0, flat.size, step):
        c = flat[i : i + step]
        if not np.all((c == 0.0) | (c == 1.0)):
            return False
    return True


def kernel(tokens, hdc_vocab, **run_kwargs):
    tok = np.ascontiguousarray(np.asarray(tokens).astype(np.int32))
    vocab = np.ascontiguousarray(np.asarray(hdc_vocab, dtype=np.float32))
    assert tok.shape == (N_TOKENS,)
    assert vocab.shape == (VOCAB, DIM)

    out_dt = mybir.dt.uint8 if _is_binary(vocab) else mybir.dt.bfloat16

    shards = tok.reshape(N_CORES, TOK_PER_CORE)
    in_maps = [{"tokens": shards[i], "hdc_vocab": vocab} for i in range(N_CORES)]
    res = run_bass_kernel_spmd(
        _get_nc(out_dt), in_maps, core_ids=list(range(N_CORES)), **run_kwargs
    )
    out = np.concatenate(
        [np.asarray(r["out"]).astype(np.float32) for r in res.results], axis=0
    )
    if run_kwargs:
        return out, res
    return out
